# revision 68
# baseline (speedup 1.0000x reference)
"""Multi-head attention Trainium2 Bass kernel.

Problem: B=4, S=2048, D=512, H=8 heads (dk=64), fp32, with int32 attention
mask and scores output. Returns (output, scores) like the reference.

Sharding: 8 cores; core c handles batch b = c//2, head-group g = c%2
(4 heads each). QKV/out projection weights are split along the head dim
(tensor-parallel); the out-projection partial sums of the two cores of a
batch are reduced on the host (cheap: 4 MiB per batch).

Device kernel (per core), all matmuls in fp32r (rounded fp32, ~1.6e-4):
  phase 1: PE-transpose q,k,v tiles; project to qhT/khT [dk,s] and vh [s,dk].
  phase 2: per (head, 128-row strip): scores via PE (K=64), exp on ACT,
           mask-mult + row-sum on DVE (scalar_tensor_tensor), then two
           branches: POOL normalizes -> DMA probs out; PE transposes the
           unnormalized probs -> attn @ v accumulation, scaled by 1/Z (as a
           row, broadcast over partitions via a small DRAM roundtrip) on the
           way out of PSUM.
  phase 3: out-projection from attnT -> outT partial, DMA out.
"""

import sys

if "/opt/trn_rl_repo" not in sys.path:
    sys.path.insert(0, "/opt/trn_rl_repo")

import numpy as np

B, S, D, H = 4, 2048, 512, 8
DK = 64          # head dim
NH = 4           # heads per core
DKG = NH * DK    # 256, head-group width
P = 128
N_CORES = 8
SM = 512         # phase-1 s-macro
QM = 256         # phase-2 q-macro rows

_CACHE = {}


def _build_nc():
    import concourse.bass as bass
    import concourse.mybir as mybir
    import concourse.tile as tile
    from concourse import bacc
    from concourse.masks import make_identity

    f32 = mybir.dt.float32
    f32r = mybir.dt.float32r
    i32 = mybir.dt.int32
    AF = mybir.ActivationFunctionType
    OP = mybir.AluOpType

    nc = bacc.Bacc("TRN2", target_bir_lowering=False, debug=False)

    q_d = nc.dram_tensor("q", [S, D], f32, kind="ExternalInput").ap()
    k_d = nc.dram_tensor("k", [S, D], f32, kind="ExternalInput").ap()
    v_d = nc.dram_tensor("v", [S, D], f32, kind="ExternalInput").ap()
    u8 = mybir.dt.uint8
    mask_d = nc.dram_tensor("mask", [S, S], u8, kind="ExternalInput").ap()
    wqt_d = nc.dram_tensor("wqt", [D, DKG], f32, kind="ExternalInput").ap()
    wkt_d = nc.dram_tensor("wkt", [D, DKG], f32, kind="ExternalInput").ap()
    wvt_d = nc.dram_tensor("wvt", [D, DKG], f32, kind="ExternalInput").ap()
    wot_d = nc.dram_tensor("wot", [DKG, D], f32, kind="ExternalInput").ap()
    bqc_d = nc.dram_tensor("bqc", [P, 2], f32, kind="ExternalInput").ap()
    bkc_d = nc.dram_tensor("bkc", [P, 2], f32, kind="ExternalInput").ap()
    bv2_d = nc.dram_tensor("bv2", [1, DKG], f32, kind="ExternalInput").ap()
    scores_d = nc.dram_tensor("scores4", [NH, S, S], f32, kind="ExternalOutput").ap()
    outt_d = nc.dram_tensor("outt", [D, S], f32, kind="ExternalOutput").ap()

    with tile.TileContext(nc) as tc:
        with tc.tile_pool(name="persist", bufs=1) as pp:
            idr = pp.tile([P, P], f32r)
            idf2 = pp.tile([P, P], f32)
            wo_r = pp.tile([DK, 4, D], f32r)
            bqc = pp.tile([P, 2], f32)
            bkc = pp.tile([P, 2], f32)
            bv2 = pp.tile([P, DKG], f32)
            nc.sync.dma_start(bqc[:], bqc_d[:])
            nc.sync.dma_start(bkc[:], bkc_d[:])
            bv2_bcast = bass.AP(
                tensor=bv2_d.tensor, offset=bv2_d.offset,
                ap=[[0, P], bv2_d.ap[1]])
            nc.gpsimd.dma_start(out=bv2[:], in_=bv2_bcast)

            # persistent activations
            qht = pp.tile([P, 2, S], f32r)   # [dk%128, dk-chunk, s]
            kht = pp.tile([P, 2, S], f32r)
            vh = pp.tile([P, 16, DKG], f32r)  # [s%128, s-tile, dkg]
            at_sb = pp.tile([DK, NH, S], f32r)  # attnT [dk%64, head, s]

            # ---------------- phase 1: transpose + projections ----------------
            with tc.tile_pool(name="p1c", bufs=1) as p1c, \
                 tc.tile_pool(name="p1", bufs=2) as p1, \
                 tc.tile_pool(name="p1t", bufs=1) as p1t, \
                 tc.tile_pool(name="p1ps", bufs=2, space="PSUM") as p1ps:
                # identities for PE transposes
                make_identity(nc, idf2[:])
                idf = idf2
                nc.scalar.activation(idr[:], idf2[:], AF.Copy)

                # weights -> SBUF, rounded to f32r
                wq_s = p1c.tile([P, 4, DKG], f32)
                wk_s = p1c.tile([P, 4, DKG], f32)
                wv_s = p1c.tile([P, 4, DKG], f32)
                wo_s = p1c.tile([DK, 4, D], f32)
                nc.sync.dma_start(wq_s[:], wqt_d.rearrange("(ko p) m -> p ko m", p=P))
                nc.sync.dma_start(wk_s[:], wkt_d.rearrange("(ko p) m -> p ko m", p=P))
                nc.sync.dma_start(wv_s[:], wvt_d.rearrange("(ko p) m -> p ko m", p=P))
                nc.sync.dma_start(wo_s[:], wot_d.rearrange("(h d) m -> d h m", h=4))
                wq_r = p1c.tile([P, 4, DKG], f32r)
                wk_r = p1c.tile([P, 4, DKG], f32r)
                wv_r = p1c.tile([P, 4, DKG], f32r)
                nc.scalar.activation(wq_r[:], wq_s[:], AF.Copy)
                nc.scalar.activation(wk_r[:], wk_s[:], AF.Copy)
                nc.scalar.activation(wv_r[:], wv_s[:], AF.Copy)
                nc.scalar.activation(wo_r[:], wo_s[:], AF.Copy)
                for m in range(S // SM):
                    raw = {}
                    for name, src in (("q", q_d), ("k", k_d), ("v", v_d)):
                        t = p1.tile([P, 4, SM], f32, tag=f"raw_{name}")
                        nc.sync.dma_start(
                            t[:],
                            src[m * SM:(m + 1) * SM, :].rearrange(
                                "(o p) d -> p o d", p=P),
                        )
                        raw[name] = t
                    tr = {}
                    for name in ("q", "k", "v"):
                        t = p1t.tile([P, 4, SM], f32r, tag=f"tr_{name}")
                        for db in range(4):
                            ps = p1ps.tile([P, SM], f32, tag="tps")
                            for o in range(4):
                                nc.tensor.transpose(
                                    ps[:, o * P:(o + 1) * P],
                                    raw[name][:, o, db * P:(db + 1) * P],
                                    idf[:],
                                )
                            nc.scalar.activation(t[:, db, :], ps[:], AF.Copy)
                        tr[name] = t
                    # q/k projections -> qht/kht [dk, s]
                    for name, w_r, bc, dst in (
                        ("q", wq_r, bqc, qht), ("k", wk_r, bkc, kht)):
                        for ch in range(2):
                            ps = p1ps.tile([P, SM], f32, tag="pproj")
                            for ko in range(4):
                                nc.tensor.matmul(
                                    ps[:],
                                    w_r[:, ko, ch * P:(ch + 1) * P],
                                    tr[name][:, ko, :],
                                    start=(ko == 0), stop=(ko == 3),
                                )
                            nc.scalar.activation(
                                dst[:, ch, m * SM:(m + 1) * SM], ps[:],
                                AF.Identity, bias=bc[:, ch:ch + 1])
                    # v projection -> vh [s, dkg]
                    for so in range(4):
                        ps = p1ps.tile([P, DKG], f32, tag="pv")
                        for ko in range(4):
                            nc.tensor.matmul(
                                ps[:],
                                tr["v"][:, ko, so * P:(so + 1) * P],
                                wv_r[:, ko, :],
                                start=(ko == 0), stop=(ko == 3),
                            )
                        nc.vector.tensor_tensor(
                            out=vh[:, m * 4 + so, :], in0=ps[:],
                            in1=bv2[:], op=OP.add)

            # ---------------- phase 2: attention ----------------
            with tc.tile_pool(name="p2", bufs=2) as p2, \
                 tc.tile_pool(name="p2three", bufs=3) as p2three, \
                 tc.tile_pool(name="p2one", bufs=1) as p2one, \
                 tc.tile_pool(name="p2s", bufs=4) as p2s, \
                 tc.tile_pool(name="psS", bufs=2, space="PSUM") as psS, \
                 tc.tile_pool(name="psT", bufs=2, space="PSUM") as psT, \
                 tc.tile_pool(name="psA", bufs=1, space="PSUM") as psA, \
                 tc.tile_pool(name="psZ", bufs=1, space="PSUM") as psZ, \
                 tc.tile_pool(name="zdram", bufs=2, space="DRAM") as zdram:
                _ptc = [0]
                for qm in range(S // QM):
                    mask_sb = p2one.tile([P, 2, S], u8, tag="mask")
                    nc.sync.dma_start(
                        mask_sb[:],
                        mask_d[qm * QM:(qm + 1) * QM, :].rearrange(
                            "(o p) s -> p o s", p=P),
                    )
                    maskf = p2one.tile([P, 2, S], f32, tag="maskf")
                    nc.gpsimd.tensor_copy(maskf[:, 0, :], mask_sb[:, 0, :])
                    nc.gpsimd.tensor_copy(maskf[:, 1, :], mask_sb[:, 1, :])
                    for h in range(NH):
                        off = (h % 2) * DK
                        ch = h // 2
                        pt_h = p2.tile([P, 16, QM], f32r, tag="pt")
                        zr2 = p2s.tile([P, 2], f32, tag="zr2")
                        for qs in range(2):
                            row = qm * QM + qs * P
                            eprob = p2.tile([P, S], f32, tag="eprob")
                            for n in range(4):
                                ps_s = psS.tile([P, 512], f32, tag="ps_s")
                                nc.tensor.matmul(
                                    ps_s[:],
                                    qht[off:off + DK, ch, row:row + P],
                                    kht[off:off + DK, ch, n * 512:(n + 1) * 512],
                                    start=True, stop=True,
                                )
                                nc.scalar.activation(
                                    eprob[:, n * 512:(n + 1) * 512],
                                    ps_s[:], AF.Exp)
                            # masked unnormalized probs (f32r) + row sums
                            pm = p2three.tile([P, S], f32r, tag="pm")
                            z = p2s.tile([P, 1], f32, tag="z")
                            nc.vector.scalar_tensor_tensor(
                                out=pm[:], in0=eprob[:], scalar=1.0,
                                in1=maskf[:, qs, :],
                                op0=OP.mult, op1=OP.mult, accum_out=z[:])
                            nc.vector.reciprocal(zr2[:, qs:qs + 1], z[:])
                            # normalized probs -> DRAM (off the attn path)
                            pn = p2.tile([P, S], f32r, tag="pn")
                            nc.gpsimd.tensor_scalar_mul(
                                out=pn[:], in0=pm[:].bitcast(f32),
                                scalar1=zr2[:, qs:qs + 1])
                            nc.sync.dma_start(
                                scores_d[h, row:row + P, :], pn[:].bitcast(f32))
                            # transpose unnormalized probs into pt_h
                            for kt8 in range(2):
                                ps = psT.tile([P, 1024], f32r, tag="ps_t")
                                for j in range(8):
                                    kt = kt8 * 8 + j
                                    nc.tensor.transpose(
                                        ps[:, j * P:(j + 1) * P],
                                        pm[:, kt * P:(kt + 1) * P],
                                        idr[:],
                                    )
                                # alternate the psum->SBUF copies DVE/ACT
                                _ptc[0] += 1
                                dst = pt_h[:, kt8 * 8:(kt8 + 1) * 8,
                                           qs * P:(qs + 1) * P]
                                src = ps[:].rearrange("p (j c) -> p j c", j=8)
                                if _ptc[0] % 4 != 3:
                                    nc.vector.tensor_copy(dst, src)
                                else:
                                    nc.scalar.activation(dst, src, AF.Copy)
                        # 1/Z as a row, broadcast over dk partitions via DRAM
                        ps_z = psZ.tile([2, P], f32, tag="ps_z")
                        nc.tensor.transpose(ps_z[:], zr2[:], idf2[:])
                        zrt = p2s.tile([2, P], f32, tag="zrt")
                        nc.scalar.activation(zrt[:], ps_z[:], AF.Copy)
                        zd = zdram.tile([2, P], f32, tag="zd")
                        nc.sync.dma_start(zd[:], zrt[:])
                        zb = p2s.tile([DK, 2, P], f32, tag="zb")
                        zd_b = bass.AP(
                            tensor=zd.tensor, offset=zd.offset,
                            ap=[[0, DK], zd.ap[0], zd.ap[1]])
                        nc.sync.dma_start(zb[:], zd_b)
                        # attn for head h: attnT[dk, q] += vh_h.T @ probsT
                        psum_a = psA.tile([DK, QM], f32, tag="ps_a")
                        for kt in range(16):
                            nc.tensor.matmul(
                                psum_a[:],
                                vh[:, kt, h * DK:(h + 1) * DK],
                                pt_h[:, kt, :],
                                start=(kt == 0), stop=(kt == 15),
                            )
                        # scale by 1/Z while copying out of PSUM
                        nc.vector.tensor_tensor(
                            out=at_sb[:, h, qm * QM:(qm + 1) * QM],
                            in0=psum_a[:],
                            in1=zb[:].rearrange("p a b -> p (a b)"),
                            op=OP.mult)

            # ---------------- phase 3: out projection ----------------
            with tc.tile_pool(name="p3", bufs=3) as p3, \
                 tc.tile_pool(name="p3ps", bufs=2, space="PSUM") as p3ps:
                for dc in range(4):
                    for n in range(4):
                        ps = p3ps.tile([P, 512], f32, tag="ps_o")
                        for ko in range(NH):
                            nc.tensor.matmul(
                                ps[:],
                                wo_r[:, ko, dc * P:(dc + 1) * P],
                                at_sb[:, ko, n * 512:(n + 1) * 512],
                                start=(ko == 0), stop=(ko == NH - 1),
                            )
                        ot = p3.tile([P, 512], f32, tag="ot")
                        nc.scalar.activation(ot[:], ps[:], AF.Copy)
                        nc.sync.dma_start(
                            outt_d[dc * P:(dc + 1) * P, n * 512:(n + 1) * 512],
                            ot[:])

    nc.compile()
    return nc


def _get_nc():
    if "nc" not in _CACHE:
        _CACHE["nc"] = _build_nc()
    return _CACHE["nc"]


def make_in_maps(q, k, v, mask, Wq, bq, Wk, bk, Wv, bv, Wo, bo):
    """Build the 8 per-core input maps (host-side sharding + weight prep)."""
    scale = 1.0 / np.sqrt(DK)
    q = np.asarray(q, np.float32)
    k = np.asarray(k, np.float32)
    v = np.asarray(v, np.float32)
    mask_u8 = np.ascontiguousarray(
        (np.asarray(mask) != 0).astype(np.uint8))
    Wq = np.asarray(Wq, np.float32)
    Wk = np.asarray(Wk, np.float32)
    Wv = np.asarray(Wv, np.float32)
    Wo = np.asarray(Wo, np.float32)
    bq = np.asarray(bq, np.float32)
    bk = np.asarray(bk, np.float32)
    bv = np.asarray(bv, np.float32)

    in_maps = []
    for c in range(N_CORES):
        b, g = divmod(c, 2)
        sl = slice(g * DKG, (g + 1) * DKG)
        wqt = np.ascontiguousarray((Wq[sl, :] * scale).T)   # [D, DKG]
        wkt = np.ascontiguousarray(Wk[sl, :].T)
        wvt = np.ascontiguousarray(Wv[sl, :].T)
        wot = np.ascontiguousarray(Wo[:, sl].T)             # [DKG, D]
        bqc = np.ascontiguousarray((bq[sl] * scale).reshape(2, P).T)  # [P, 2]
        bkc = np.ascontiguousarray(bk[sl].reshape(2, P).T)
        bv2 = np.ascontiguousarray(bv[sl].reshape(1, DKG))
        in_maps.append({
            "q": np.ascontiguousarray(q[b]),
            "k": np.ascontiguousarray(k[b]),
            "v": np.ascontiguousarray(v[b]),
            "mask": mask_u8[b],
            "wqt": wqt, "wkt": wkt, "wvt": wvt, "wot": wot,
            "bqc": bqc, "bkc": bkc, "bv2": bv2,
        })
    return in_maps


def assemble(results, bo):
    """Gather per-core results into (output, scores)."""
    bo = np.asarray(bo, np.float32)
    scores = np.empty((B, H, S, S), np.float32)
    output = np.empty((B, S, D), np.float32)
    for c in range(N_CORES):
        b, g = divmod(c, 2)
        scores[b, g * NH:(g + 1) * NH] = results[c]["scores4"]
    for b in range(B):
        output[b] = (results[2 * b]["outt"] + results[2 * b + 1]["outt"]).T + bo
    return output, scores


def kernel(q, k, v, mask, Wq, bq, Wk, bk, Wv, bv, Wo, bo):
    from concourse.bass_utils import run_bass_kernel_spmd

    nc = _get_nc()
    in_maps = make_in_maps(q, k, v, mask, Wq, bq, Wk, bk, Wv, bv, Wo, bo)
    res = run_bass_kernel_spmd(nc, in_maps, core_ids=list(range(N_CORES)))
    return assemble(res.results, bo)



# revision 71
# speedup vs baseline: 1.0050x; 1.0050x over previous
"""Multi-head attention Trainium2 Bass kernel.

Problem: B=4, S=2048, D=512, H=8 heads (dk=64), fp32, with int32 attention
mask and scores output. Returns (output, scores) like the reference.

Sharding: 8 cores; core c handles batch b = c//2, head-group g = c%2
(4 heads each). QKV/out projection weights are split along the head dim
(tensor-parallel); the out-projection partial sums of the two cores of a
batch are reduced on the host (cheap: 4 MiB per batch).

Device kernel (per core), all matmuls in fp32r (rounded fp32, ~1.6e-4):
  phase 1: PE-transpose q,k,v tiles; project to qhT/khT [dk,s] and vh [s,dk].
  phase 2: per (head, 128-row strip): scores via PE (K=64), exp on ACT,
           mask-mult + row-sum on DVE (scalar_tensor_tensor), then two
           branches: POOL normalizes -> DMA probs out; PE transposes the
           unnormalized probs -> attn @ v accumulation, scaled by 1/Z (as a
           row, broadcast over partitions via a small DRAM roundtrip) on the
           way out of PSUM.
  phase 3: out-projection from attnT -> outT partial, DMA out.
"""

import sys

if "/opt/trn_rl_repo" not in sys.path:
    sys.path.insert(0, "/opt/trn_rl_repo")

import numpy as np

B, S, D, H = 4, 2048, 512, 8
DK = 64          # head dim
NH = 4           # heads per core
DKG = NH * DK    # 256, head-group width
P = 128
N_CORES = 8
SM = 512         # phase-1 s-macro
QM = 256         # phase-2 q-macro rows

_CACHE = {}


def _build_nc():
    import concourse.bass as bass
    import concourse.mybir as mybir
    import concourse.tile as tile
    from concourse import bacc
    from concourse.masks import make_identity

    f32 = mybir.dt.float32
    f32r = mybir.dt.float32r
    i32 = mybir.dt.int32
    AF = mybir.ActivationFunctionType
    OP = mybir.AluOpType

    nc = bacc.Bacc("TRN2", target_bir_lowering=False, debug=False)

    q_d = nc.dram_tensor("q", [S, D], f32, kind="ExternalInput").ap()
    k_d = nc.dram_tensor("k", [S, D], f32, kind="ExternalInput").ap()
    v_d = nc.dram_tensor("v", [S, D], f32, kind="ExternalInput").ap()
    u8 = mybir.dt.uint8
    mask_d = nc.dram_tensor("mask", [S, S], u8, kind="ExternalInput").ap()
    wqt_d = nc.dram_tensor("wqt", [D, DKG], f32, kind="ExternalInput").ap()
    wkt_d = nc.dram_tensor("wkt", [D, DKG], f32, kind="ExternalInput").ap()
    wvt_d = nc.dram_tensor("wvt", [D, DKG], f32, kind="ExternalInput").ap()
    wot_d = nc.dram_tensor("wot", [DKG, D], f32, kind="ExternalInput").ap()
    bqc_d = nc.dram_tensor("bqc", [P, 2], f32, kind="ExternalInput").ap()
    bkc_d = nc.dram_tensor("bkc", [P, 2], f32, kind="ExternalInput").ap()
    bv2_d = nc.dram_tensor("bv2", [1, DKG], f32, kind="ExternalInput").ap()
    scores_d = nc.dram_tensor("scores4", [NH, S, S], f32, kind="ExternalOutput").ap()
    outt_d = nc.dram_tensor("outt", [D, S], f32, kind="ExternalOutput").ap()

    with tile.TileContext(nc) as tc:
        with tc.tile_pool(name="persist", bufs=1) as pp:
            idr = pp.tile([P, P], f32r)
            idf2 = pp.tile([P, P], f32)
            wo_r = pp.tile([DK, 4, D], f32r)
            bqc = pp.tile([P, 2], f32)
            bkc = pp.tile([P, 2], f32)
            bv2 = pp.tile([P, DKG], f32)
            nc.sync.dma_start(bqc[:], bqc_d[:])
            nc.sync.dma_start(bkc[:], bkc_d[:])
            bv2_bcast = bass.AP(
                tensor=bv2_d.tensor, offset=bv2_d.offset,
                ap=[[0, P], bv2_d.ap[1]])
            nc.gpsimd.dma_start(out=bv2[:], in_=bv2_bcast)

            # persistent activations
            qht = pp.tile([P, 2, S], f32r)   # [dk%128, dk-chunk, s]
            kht = pp.tile([P, 2, S], f32r)
            vh = pp.tile([P, 16, DKG], f32r)  # [s%128, s-tile, dkg]
            at_sb = pp.tile([DK, NH, S], f32r)  # attnT [dk%64, head, s]

            # ---------------- phase 1: transpose + projections ----------------
            with tc.tile_pool(name="p1c", bufs=1) as p1c, \
                 tc.tile_pool(name="p1", bufs=2) as p1, \
                 tc.tile_pool(name="p1t", bufs=1) as p1t, \
                 tc.tile_pool(name="p1ps", bufs=2, space="PSUM") as p1ps:
                # identities for PE transposes
                make_identity(nc, idf2[:])
                idf = idf2
                nc.scalar.activation(idr[:], idf2[:], AF.Copy)

                # weights -> SBUF, rounded to f32r
                wq_s = p1c.tile([P, 4, DKG], f32)
                wk_s = p1c.tile([P, 4, DKG], f32)
                wv_s = p1c.tile([P, 4, DKG], f32)
                wo_s = p1c.tile([DK, 4, D], f32)
                nc.sync.dma_start(wq_s[:], wqt_d.rearrange("(ko p) m -> p ko m", p=P))
                nc.sync.dma_start(wk_s[:], wkt_d.rearrange("(ko p) m -> p ko m", p=P))
                nc.sync.dma_start(wv_s[:], wvt_d.rearrange("(ko p) m -> p ko m", p=P))
                nc.sync.dma_start(wo_s[:], wot_d.rearrange("(h d) m -> d h m", h=4))
                wq_r = p1c.tile([P, 4, DKG], f32r)
                wk_r = p1c.tile([P, 4, DKG], f32r)
                wv_r = p1c.tile([P, 4, DKG], f32r)
                nc.scalar.activation(wq_r[:], wq_s[:], AF.Copy)
                nc.scalar.activation(wk_r[:], wk_s[:], AF.Copy)
                nc.scalar.activation(wv_r[:], wv_s[:], AF.Copy)
                nc.scalar.activation(wo_r[:], wo_s[:], AF.Copy)
                for m in range(S // SM):
                    raw = {}
                    for name, src in (("q", q_d), ("k", k_d), ("v", v_d)):
                        t = p1.tile([P, 4, SM], f32, tag=f"raw_{name}")
                        nc.sync.dma_start(
                            t[:],
                            src[m * SM:(m + 1) * SM, :].rearrange(
                                "(o p) d -> p o d", p=P),
                        )
                        raw[name] = t
                    tr = {}
                    for name in ("q", "k", "v"):
                        t = p1t.tile([P, 4, SM], f32r, tag=f"tr_{name}")
                        for db in range(4):
                            ps = p1ps.tile([P, SM], f32, tag="tps")
                            for o in range(4):
                                nc.tensor.transpose(
                                    ps[:, o * P:(o + 1) * P],
                                    raw[name][:, o, db * P:(db + 1) * P],
                                    idf[:],
                                )
                            nc.scalar.activation(t[:, db, :], ps[:], AF.Copy)
                        tr[name] = t
                    # q/k projections -> qht/kht [dk, s]
                    for name, w_r, bc, dst in (
                        ("q", wq_r, bqc, qht), ("k", wk_r, bkc, kht)):
                        for ch in range(2):
                            ps = p1ps.tile([P, SM], f32, tag="pproj")
                            for ko in range(4):
                                nc.tensor.matmul(
                                    ps[:],
                                    w_r[:, ko, ch * P:(ch + 1) * P],
                                    tr[name][:, ko, :],
                                    start=(ko == 0), stop=(ko == 3),
                                )
                            nc.scalar.activation(
                                dst[:, ch, m * SM:(m + 1) * SM], ps[:],
                                AF.Identity, bias=bc[:, ch:ch + 1])
                    # v projection -> vh [s, dkg]
                    for so in range(4):
                        ps = p1ps.tile([P, DKG], f32, tag="pv")
                        for ko in range(4):
                            nc.tensor.matmul(
                                ps[:],
                                tr["v"][:, ko, so * P:(so + 1) * P],
                                wv_r[:, ko, :],
                                start=(ko == 0), stop=(ko == 3),
                            )
                        nc.vector.tensor_tensor(
                            out=vh[:, m * 4 + so, :], in0=ps[:],
                            in1=bv2[:], op=OP.add)

            # ---------------- phase 2: attention ----------------
            with tc.tile_pool(name="p2", bufs=2) as p2, \
                 tc.tile_pool(name="p2three", bufs=3) as p2three, \
                 tc.tile_pool(name="p2one", bufs=1) as p2one, \
                 tc.tile_pool(name="p2s", bufs=4) as p2s, \
                 tc.tile_pool(name="psS", bufs=2, space="PSUM") as psS, \
                 tc.tile_pool(name="psT", bufs=2, space="PSUM") as psT, \
                 tc.tile_pool(name="psA", bufs=1, space="PSUM") as psA, \
                 tc.tile_pool(name="psZ", bufs=1, space="PSUM") as psZ, \
                 tc.tile_pool(name="zdram", bufs=2, space="DRAM") as zdram:
                _ptc = [0]
                for qm in range(S // QM):
                    mask_sb = p2one.tile([P, 2, S], u8, tag="mask")
                    nc.sync.dma_start(
                        mask_sb[:],
                        mask_d[qm * QM:(qm + 1) * QM, :].rearrange(
                            "(o p) s -> p o s", p=P),
                    )
                    maskf = p2.tile([P, 2, S], f32, tag="maskf")
                    nc.gpsimd.tensor_copy(maskf[:, 0, :], mask_sb[:, 0, :])
                    nc.gpsimd.tensor_copy(maskf[:, 1, :], mask_sb[:, 1, :])
                    for h in range(NH):
                        off = (h % 2) * DK
                        ch = h // 2
                        pt_h = p2one.tile([P, 16, QM], f32r, tag="pt")
                        zr2 = p2s.tile([P, 2], f32, tag="zr2")
                        for qs in range(2):
                            row = qm * QM + qs * P
                            eprob = p2.tile([P, S], f32, tag="eprob")
                            for n in range(4):
                                ps_s = psS.tile([P, 512], f32, tag="ps_s")
                                nc.tensor.matmul(
                                    ps_s[:],
                                    qht[off:off + DK, ch, row:row + P],
                                    kht[off:off + DK, ch, n * 512:(n + 1) * 512],
                                    start=True, stop=True,
                                )
                                nc.scalar.activation(
                                    eprob[:, n * 512:(n + 1) * 512],
                                    ps_s[:], AF.Exp)
                            # masked unnormalized probs (f32r) + row sums
                            pm = p2three.tile([P, S], f32r, tag="pm")
                            z = p2s.tile([P, 1], f32, tag="z")
                            nc.vector.scalar_tensor_tensor(
                                out=pm[:], in0=eprob[:], scalar=1.0,
                                in1=maskf[:, qs, :],
                                op0=OP.mult, op1=OP.mult, accum_out=z[:])
                            nc.vector.reciprocal(zr2[:, qs:qs + 1], z[:])
                            # normalized probs -> DRAM (off the attn path)
                            pn = p2.tile([P, S], f32r, tag="pn")
                            nc.gpsimd.tensor_scalar_mul(
                                out=pn[:], in0=pm[:].bitcast(f32),
                                scalar1=zr2[:, qs:qs + 1])
                            nc.sync.dma_start(
                                scores_d[h, row:row + P, :], pn[:].bitcast(f32))
                            # transpose unnormalized probs into pt_h
                            for kt8 in range(2):
                                ps = psT.tile([P, 1024], f32r, tag="ps_t")
                                for j in range(8):
                                    kt = kt8 * 8 + j
                                    nc.tensor.transpose(
                                        ps[:, j * P:(j + 1) * P],
                                        pm[:, kt * P:(kt + 1) * P],
                                        idr[:],
                                    )
                                # alternate the psum->SBUF copies DVE/ACT
                                _ptc[0] += 1
                                dst = pt_h[:, kt8 * 8:(kt8 + 1) * 8,
                                           qs * P:(qs + 1) * P]
                                src = ps[:].rearrange("p (j c) -> p j c", j=8)
                                if _ptc[0] % 4 != 3:
                                    nc.vector.tensor_copy(dst, src)
                                else:
                                    nc.scalar.activation(dst, src, AF.Copy)
                        # 1/Z as a row, broadcast over dk partitions via DRAM
                        ps_z = psZ.tile([2, P], f32, tag="ps_z")
                        nc.tensor.transpose(ps_z[:], zr2[:], idf2[:])
                        zrt = p2s.tile([2, P], f32, tag="zrt")
                        nc.scalar.activation(zrt[:], ps_z[:], AF.Copy)
                        zd = zdram.tile([2, P], f32, tag="zd")
                        nc.sync.dma_start(zd[:], zrt[:])
                        zb = p2s.tile([DK, 2, P], f32, tag="zb")
                        zd_b = bass.AP(
                            tensor=zd.tensor, offset=zd.offset,
                            ap=[[0, DK], zd.ap[0], zd.ap[1]])
                        nc.sync.dma_start(zb[:], zd_b)
                        # attn for head h: attnT[dk, q] += vh_h.T @ probsT
                        psum_a = psA.tile([DK, QM], f32, tag="ps_a")
                        for kt in range(16):
                            nc.tensor.matmul(
                                psum_a[:],
                                vh[:, kt, h * DK:(h + 1) * DK],
                                pt_h[:, kt, :],
                                start=(kt == 0), stop=(kt == 15),
                            )
                        # scale by 1/Z while copying out of PSUM
                        nc.vector.tensor_tensor(
                            out=at_sb[:, h, qm * QM:(qm + 1) * QM],
                            in0=psum_a[:],
                            in1=zb[:].rearrange("p a b -> p (a b)"),
                            op=OP.mult)

            # ---------------- phase 3: out projection ----------------
            with tc.tile_pool(name="p3", bufs=3) as p3, \
                 tc.tile_pool(name="p3ps", bufs=2, space="PSUM") as p3ps:
                for dc in range(4):
                    for n in range(4):
                        ps = p3ps.tile([P, 512], f32, tag="ps_o")
                        for ko in range(NH):
                            nc.tensor.matmul(
                                ps[:],
                                wo_r[:, ko, dc * P:(dc + 1) * P],
                                at_sb[:, ko, n * 512:(n + 1) * 512],
                                start=(ko == 0), stop=(ko == NH - 1),
                            )
                        ot = p3.tile([P, 512], f32, tag="ot")
                        nc.scalar.activation(ot[:], ps[:], AF.Copy)
                        nc.sync.dma_start(
                            outt_d[dc * P:(dc + 1) * P, n * 512:(n + 1) * 512],
                            ot[:])

    nc.compile()
    return nc


def _get_nc():
    if "nc" not in _CACHE:
        _CACHE["nc"] = _build_nc()
    return _CACHE["nc"]


def make_in_maps(q, k, v, mask, Wq, bq, Wk, bk, Wv, bv, Wo, bo):
    """Build the 8 per-core input maps (host-side sharding + weight prep)."""
    scale = 1.0 / np.sqrt(DK)
    q = np.asarray(q, np.float32)
    k = np.asarray(k, np.float32)
    v = np.asarray(v, np.float32)
    mask_u8 = np.ascontiguousarray(
        (np.asarray(mask) != 0).astype(np.uint8))
    Wq = np.asarray(Wq, np.float32)
    Wk = np.asarray(Wk, np.float32)
    Wv = np.asarray(Wv, np.float32)
    Wo = np.asarray(Wo, np.float32)
    bq = np.asarray(bq, np.float32)
    bk = np.asarray(bk, np.float32)
    bv = np.asarray(bv, np.float32)

    in_maps = []
    for c in range(N_CORES):
        b, g = divmod(c, 2)
        sl = slice(g * DKG, (g + 1) * DKG)
        wqt = np.ascontiguousarray((Wq[sl, :] * scale).T)   # [D, DKG]
        wkt = np.ascontiguousarray(Wk[sl, :].T)
        wvt = np.ascontiguousarray(Wv[sl, :].T)
        wot = np.ascontiguousarray(Wo[:, sl].T)             # [DKG, D]
        bqc = np.ascontiguousarray((bq[sl] * scale).reshape(2, P).T)  # [P, 2]
        bkc = np.ascontiguousarray(bk[sl].reshape(2, P).T)
        bv2 = np.ascontiguousarray(bv[sl].reshape(1, DKG))
        in_maps.append({
            "q": np.ascontiguousarray(q[b]),
            "k": np.ascontiguousarray(k[b]),
            "v": np.ascontiguousarray(v[b]),
            "mask": mask_u8[b],
            "wqt": wqt, "wkt": wkt, "wvt": wvt, "wot": wot,
            "bqc": bqc, "bkc": bkc, "bv2": bv2,
        })
    return in_maps


def assemble(results, bo):
    """Gather per-core results into (output, scores)."""
    bo = np.asarray(bo, np.float32)
    scores = np.empty((B, H, S, S), np.float32)
    output = np.empty((B, S, D), np.float32)
    for c in range(N_CORES):
        b, g = divmod(c, 2)
        scores[b, g * NH:(g + 1) * NH] = results[c]["scores4"]
    for b in range(B):
        output[b] = (results[2 * b]["outt"] + results[2 * b + 1]["outt"]).T + bo
    return output, scores


def kernel(q, k, v, mask, Wq, bq, Wk, bk, Wv, bv, Wo, bo):
    from concourse.bass_utils import run_bass_kernel_spmd

    nc = _get_nc()
    in_maps = make_in_maps(q, k, v, mask, Wq, bq, Wk, bk, Wv, bv, Wo, bo)
    res = run_bass_kernel_spmd(nc, in_maps, core_ids=list(range(N_CORES)))
    return assemble(res.results, bo)

